# revision 33
# baseline (speedup 1.0000x reference)
"""Trainium2 Bass kernel for a dense transformer block (pre-LN, single-head
attention + GELU MLP), data-parallel over the batch dim across 8 NeuronCores.

Per-core problem (batch element): x [S=2048, D=512]
    h  = LN(x; g1, b1)
    q, k, v = h @ wq, h @ wk, h @ wv
    scores = q @ k.T / D ; attn = softmax(scores)
    x = x + (attn @ v) @ wp
    h2 = LN(x; g2, b2)
    out = x + gelu(h2 @ w1) @ w2

v2 schedule (HW-calibrated: fp8 DR = 2x bf16 at FD=512, act-table
load = 1.3us, PE HAM re-throttles after ~3.4us idle; fp32r matmuls
compile but return garbage on HW -- do not use):

 - Two act-table phases per block: phase A (LN1+QKV+attention+LN2 stats)
   runs entirely on table set 6 (natural_log_exp_and_others: exp, ln,
   square, copy) preloaded manually; rstd = exp(-0.5*ln(var+eps)).
   Phase B (all four chunks' MLPs, deferred) uses set 10 (gelu_and_others).
   2 table loads per block instead of 16.
 - LN stat sums: s1/s2 = paired-ones @ {x8, sq8} in fp8 DoubleRow, with
   the fp8 copies/squares made on ACT.  rstd chains run 1-2 chunks ahead
   of their PE consumers so bcast matmuls never sit behind queue latency.
 - Attention steady state: chunk ch's scores/exp loop carries the
   previous chunk's tail (attnV at slots 4-11, proj/x8 at 14/15) and the
   chunk-before-that's LN2 chain (stats@1, rstd@6, finish@12), so no PE
   matmul ever sits directly behind ACT/DVE queue latency and HAM never
   re-throttles.  The epilogue interleaves the last tails with the first
   two MLP1 groups.
 - All gelus carry an explicit dep on the last phase-A ACT op: the tile
   scheduler may hoist MLP matmuls into attention bubbles (bounded by the
   2-slot mp PSUM group) but the ACT stream keeps its 2-load table plan.
   Without the pin the scheduler interleaves gelus among exps: 23 table
   loads and +130us measured on HW.
 - PSUM banks: scp/qkv/proj/stat tiles share a 3-slot group, mp 2 slots,
   dps 1 bank, u/m2 [P,2,CW] half-blobs 2 banks -> 8 exactly.
"""

import sys

for _p in ("/opt/trn_rl_repo",):
    if _p not in sys.path:
        sys.path.insert(0, _p)

from contextlib import ExitStack

import ml_dtypes
import numpy as np

import concourse.bass as bass
import concourse.tile as tile
from concourse import bacc, mybir
from concourse._compat import with_exitstack
from concourse.bass_utils import run_bass_kernel_spmd

P = 128
N_CORES = 8
FP32 = mybir.dt.float32
BF16 = mybir.dt.bfloat16
FP8 = mybir.dt.float8e4
EPS = 1e-5
DR = mybir.MatmulPerfMode.DoubleRow
AF = mybir.ActivationFunctionType

SET_LN_EXP = 6    # natural_log_exp_and_others: exp, ln, square, copy
OPTS = {}


@with_exitstack
def _block_kernel(ctx: ExitStack, tc: tile.TileContext, t, S, D, H,
                  repeat=None, trivial_gb=False):
    """t: dict of dram APs. S tokens, D model dim, H hidden dim."""
    nc = tc.nc
    DC = D // P          # feature chunks (4)
    HC = H // P          # hidden chunks (16)
    SB = S // P          # token blocks (16)
    CW = 512             # free-dim chunk width (matmul N / psum bank)
    NCH = S // CW        # token chunks (4)
    SBC = SB // NCH      # token blocks per chunk (4)

    singles = ctx.enter_context(tc.tile_pool(name="singles", bufs=1))
    e8p = ctx.enter_context(tc.tile_pool(name="e8p", bufs=2))
    gtp = ctx.enter_context(tc.tile_pool(name="gtp", bufs=2))
    avp = ctx.enter_context(tc.tile_pool(name="avp", bufs=2))
    x8p = ctx.enter_context(tc.tile_pool(name="x8p", bufs=2))
    sqp = ctx.enter_context(tc.tile_pool(name="sqp", bufs=2))
    small = ctx.enter_context(tc.tile_pool(name="small", bufs=3))
    ps = ctx.enter_context(tc.tile_pool(name="ps", bufs=3, space="PSUM"))
    psm = ctx.enter_context(tc.tile_pool(name="psm", bufs=2, space="PSUM"))
    psd = ctx.enter_context(tc.tile_pool(name="psd", bufs=1, space="PSUM"))
    psb = ctx.enter_context(tc.tile_pool(name="psb", bufs=1, space="PSUM"))

    # ---- persistent SBUF tensors ----
    x_sb = singles.tile([P, DC, S], FP32)                 # residual (feature-major)
    wq_sb = singles.tile([P, DC, D], FP8)
    wk_sb = singles.tile([P, DC, D], FP8)
    wv_sb = singles.tile([P, DC, D], FP8)
    wp_sb = singles.tile([P, DC, D], FP8)
    w1_sb = singles.tile([P, DC, H], BF16)
    w2_sb = singles.tile([P, HC, D], BF16)
    g1_sb = singles.tile([P, DC], FP32)
    b1_sb = singles.tile([P, DC], FP32)
    g2_sb = singles.tile([P, DC], FP32)
    b2_sb = singles.tile([P, DC], FP32)
    qT = singles.tile([P, DC, S], FP8)
    kT = singles.tile([P, DC, S], FP8)
    h1 = singles.tile([P, DC, S], FP8)
    v_sb = singles.tile([P, SB, D], FP8)                  # token-major V
    h2_all = singles.tile([P, DC, S], BF16)

    ones1_b = singles.tile([1, P], BF16)
    ones8p = singles.tile([P, 2, 16], FP8)               # paired ones (DR lhsT)
    eps1 = singles.tile([1, 1], FP32)
    nc.vector.memset(ones1_b, 1.0)
    nc.vector.memset(ones8p, 1.0)
    nc.vector.memset(eps1, EPS)

    # ---- load inputs (x chunked so LN1 starts early) ----
    xv = t["xT"].rearrange("(c p) s -> p c s", p=P)
    for c in range(DC):
        nc.sync.dma_start(x_sb[:, c:c + 1, 0:CW], xv[:, c:c + 1, 0:CW])
    for ch in range(1, NCH):
        sl = slice(ch * CW, (ch + 1) * CW)
        nc.sync.dma_start(x_sb[:, :, sl], xv[:, :, sl])
    nc.sync.dma_start(wk_sb, t["wk"].rearrange("(c p) m -> p c m", p=P))
    nc.sync.dma_start(wq_sb, t["wq"].rearrange("(c p) m -> p c m", p=P))
    nc.sync.dma_start(wv_sb, t["wv"].rearrange("(c p) m -> p c m", p=P))
    nc.sync.dma_start(g1_sb, t["g1"].rearrange("(c p) -> p c", p=P))
    nc.sync.dma_start(b1_sb, t["b1"].rearrange("(c p) -> p c", p=P))
    nc.sync.dma_start(g2_sb, t["g2"].rearrange("(c p) -> p c", p=P))
    nc.sync.dma_start(b2_sb, t["b2"].rearrange("(c p) -> p c", p=P))
    nc.sync.dma_start(wp_sb, t["wp"].rearrange("(c p) m -> p c m", p=P))
    nc.sync.dma_start(w1_sb, t["w1"].rearrange("(c p) m -> p c m", p=P))
    nc.sync.dma_start(w2_sb, t["w2"].rearrange("(c p) m -> p c m", p=P))

    def stats(ch, x8=None):
        """LN stat sums for token chunk ch + DVE chain to (mu, var).
        s1/s2 via fp8 DoubleRow on x8/sq8; when no x8 is supplied (LN1),
        the fp8 copy of x is made here on DVE.  Returns (mu, a_t)."""
        sl = slice(ch * CW, (ch + 1) * CW)
        s1 = ps.tile([1, CW], FP32, name="s1ps", tag="ps")
        s2 = ps.tile([1, CW], FP32, name="s2ps", tag="ps")
        if x8 is None:
            x8 = x8p.tile([P, DC, CW], FP8, name="x8")
            for c in range(DC):
                nc.scalar.copy(x8[:, c, :], x_sb[:, c, sl])
        sq8 = sqp.tile([P, DC, CW], FP8, name="sq8")
        for c in range(DC):
            nc.scalar.activation(sq8[:, c, :], x_sb[:, c, sl], AF.Square)
        for c2 in range(DC // 2):
            nc.tensor.matmul(s1, ones8p[:, :, 0:1],
                             x8[:, 2 * c2:2 * c2 + 2, :],
                             start=(c2 == 0), stop=(c2 == DC // 2 - 1),
                             skip_group_check=True, perf_mode=DR)
        for c2 in range(DC // 2):
            nc.tensor.matmul(s2, ones8p[:, :, 0:1],
                             sq8[:, 2 * c2:2 * c2 + 2, :],
                             start=(c2 == 0), stop=(c2 == DC // 2 - 1),
                             skip_group_check=True, perf_mode=DR)
        a_t = small.tile([1, CW], FP32, name="a_t", bufs=2)
        b_t = small.tile([1, CW], FP32, name="b_t", bufs=1)
        mu = small.tile([1, CW], FP32, name="mut", bufs=2)
        nc.vector.tensor_scalar_mul(mu, s1, 1.0 / D)              # mu
        nc.vector.tensor_scalar_mul(a_t, s2, 1.0 / D)             # E[x^2]
        nc.vector.tensor_mul(b_t, mu, mu)                         # mu^2
        nc.vector.tensor_tensor(a_t, a_t, b_t, mybir.AluOpType.subtract)
        return mu, a_t

    act_a_last = [None]

    def rstd(st):
        """rstd = exp(-0.5*ln(var+eps)); B = mu*rstd.  Stays on set 6."""
        mu, a_t = st
        a16 = small.tile([1, CW], BF16, name="a16", bufs=2)
        b16 = small.tile([1, CW], BF16, name="b16", bufs=2)
        nc.scalar.activation(a_t, a_t, AF.Ln, bias=eps1)
        act_a_last[0] = nc.scalar.activation(a16, a_t, AF.Exp, scale=-0.5)
        nc.vector.tensor_mul(b16, mu, a16)
        return a16, b16

    def ln_finish(ch, ab, g_sb, b_sb, dst):
        """Broadcast (a,B) and normalize chunk ch into dst."""
        a16, b16 = ab
        sl = slice(ch * CW, (ch + 1) * CW)
        a_b = ps.tile([P, CW], FP32, name="abps", tag="ps")
        b_b = ps.tile([P, CW], FP32, name="bbps", tag="ps")
        nc.tensor.matmul(a_b, ones1_b, a16, start=True, stop=True)
        nc.tensor.matmul(b_b, ones1_b, b16, start=True, stop=True)
        for c in range(DC):
            dc = dst[:, c, :]
            nc.vector.tensor_mul(dc, x_sb[:, c, sl], a_b)
            nc.vector.tensor_tensor(dc, dc, b_b, mybir.AluOpType.subtract)
            if not trivial_gb:
                nc.vector.tensor_scalar(dc, dc,
                                        g_sb[:, c:c + 1], b_sb[:, c:c + 1],
                                        mybir.AluOpType.mult,
                                        mybir.AluOpType.add)

    def qkv_k(ch):
        """k (feature-major, fp8) for chunk ch; psum->sbuf copies on ACT."""
        sl = slice(ch * CW, (ch + 1) * CW)
        for m in range(DC):
            msl = slice(m * P, (m + 1) * P)
            kp = ps.tile([P, CW], FP32, name="kps", tag="ps")
            for c2 in range(DC // 2):
                cs = slice(2 * c2, 2 * c2 + 2)
                nc.tensor.matmul(kp, wk_sb[:, cs, msl], h1[:, cs, sl],
                                 start=(c2 == 0), stop=(c2 == DC // 2 - 1),
                                 skip_group_check=True, perf_mode=DR)
            nc.scalar.copy(kT[:, m, sl], kp)

    def qkv_qv(ch):
        """q (feature-major) and v (token-major) for chunk ch."""
        sl = slice(ch * CW, (ch + 1) * CW)
        for m in range(DC):
            msl = slice(m * P, (m + 1) * P)
            qp = ps.tile([P, CW], FP32, name="qps", tag="ps")
            for c2 in range(DC // 2):
                cs = slice(2 * c2, 2 * c2 + 2)
                nc.tensor.matmul(qp, wq_sb[:, cs, msl], h1[:, cs, sl],
                                 start=(c2 == 0), stop=(c2 == DC // 2 - 1),
                                 skip_group_check=True, perf_mode=DR)
            nc.scalar.copy(qT[:, m, sl], qp)
        for sb_i in range(ch * SBC, (ch + 1) * SBC):
            tsl = slice(sb_i * P, (sb_i + 1) * P)
            vp = ps.tile([P, D], FP32, name="vps", tag="ps")
            for c2 in range(DC // 2):
                cs = slice(2 * c2, 2 * c2 + 2)
                nc.tensor.matmul(vp, h1[:, cs, tsl], wv_sb[:, cs, :],
                                 start=(c2 == 0), stop=(c2 == DC // 2 - 1),
                                 perf_mode=DR)
            nc.vector.tensor_copy(v_sb[:, sb_i, :], vp)

    def one_block(chain):
        # preload the combined ln+exp act table so the greedy inserter never
        # cycles between the exp-only and ln-only sets inside phase A (the
        # gelu set is live at block entry in repeat mode)
        nc.scalar.add_instruction(mybir.InstLoadActFuncSet(
            name=nc.get_next_instruction_name(), ins=[], outs=[],
            act_func_set_id=SET_LN_EXP))
        # ================= phase A: LN1 + QKV =================
        # loose interleave: stats run 2 chunks ahead and rstd 1 ahead of the
        # finish+qkv consumer, so the PE never sits directly behind the
        # ACT/DVE queue latency of its own chunk's chain.
        sts = [None] * NCH
        abs_ = [None] * NCH

        def fin(ch):
            ln_finish(ch, abs_[ch], g1_sb, b1_sb,
                      h1[:, :, ch * CW:(ch + 1) * CW])

        sts[0] = stats(0)
        sts[1] = stats(1)
        abs_[0] = rstd(sts[0])
        sts[2] = stats(2)
        fin(0)
        abs_[1] = rstd(sts[1])
        qkv_k(0)
        sts[3] = stats(3)
        fin(1)
        abs_[2] = rstd(sts[2])
        qkv_qv(0)
        qkv_k(1)
        fin(2)
        abs_[3] = rstd(sts[3])
        qkv_qv(1)
        qkv_k(2)
        fin(3)
        qkv_qv(2)
        qkv_k(3)
        qkv_qv(3)

        # ============ phase A: attention, chunk-pipelined ============
        class Prev:
            pass

        prev = None

        def prev_tail(pv, step):
            """One step of the previous chunk's tail, called from inside
            the current chunk's scores loop (or the epilogue)."""
            sl = slice(pv.ch * CW, (pv.ch + 1) * CW)
            if step == 2:      # rbp broadcast (rec16 ready via DVE)
                pv.rbp = ps.tile([P, CW], FP32, name="rbp", tag="ps")
                nc.tensor.matmul(pv.rbp, ones1_b, pv.rec16,
                                 start=True, stop=True)
            elif step == 3:    # rb psum->sbuf (ACT)
                pv.rb_sb = small.tile([P, CW], FP32, name="rbs", bufs=1)
                nc.scalar.copy(pv.rb_sb, pv.rbp)
            elif 4 <= step <= 11:  # attnV: two half-blob passes of 16 MMs
                half = (step - 4) // 4          # 0: m in {0,1}, 1: m in {2,3}
                qs = (step - 4) % 4             # quarter of the s2 loop
                if qs == 0:
                    pv.u = psb.tile([P, 2, CW], FP32, name="ups", tag="psb")
                for s2 in range(qs * (SB // 8), (qs + 1) * (SB // 8)):
                    for mh in range(2):
                        m = 2 * half + mh
                        nc.tensor.matmul(
                            pv.u[:, mh, :],
                            v_sb[:, 2 * s2:2 * s2 + 2, m * P:(m + 1) * P],
                            pv.e8[:, 2 * s2:2 * s2 + 2, :],
                            start=(s2 == 0), stop=(s2 == SB // 2 - 1),
                            skip_group_check=True, perf_mode=DR)
                if qs == 3:
                    for mh in range(2):
                        m = 2 * half + mh
                        nc.vector.tensor_mul(pv.avT[:, m, :],
                                             pv.u[:, mh, :], pv.rb_sb)
            elif step == 14:   # proj + residual
                for m in range(DC):
                    msl = slice(m * P, (m + 1) * P)
                    pp = ps.tile([P, CW], FP32, name="pps", tag="ps")
                    for c2 in range(DC // 2):
                        cs = slice(2 * c2, 2 * c2 + 2)
                        nc.tensor.matmul(pp, wp_sb[:, cs, msl],
                                         pv.avT[:, cs, :],
                                         start=(c2 == 0),
                                         stop=(c2 == DC // 2 - 1),
                                         perf_mode=DR)
                    nc.vector.tensor_add(x_sb[:, m, sl], x_sb[:, m, sl], pp)
            elif step == 15:   # fp8 copy of x for LN2 s1 (ACT)
                pv.x8 = x8p.tile([P, DC, CW], FP8, name="x8")
                for c in range(DC):
                    nc.scalar.copy(pv.x8[:, c, :], x_sb[:, c, sl])

        def tail2(pv, step):
            """LN2 chain for chunk pv.ch, run one further loop later so the
            serial stats->rstd->finish chain never stalls the PE."""
            if step == 0:
                pv.st2 = stats(pv.ch, x8=pv.x8)
            elif step == 1:
                pv.ab2 = rstd(pv.st2)
            elif step == 2:
                ln_finish(pv.ch, pv.ab2, g2_sb, b2_sb,
                          h2_all[:, :, pv.ch * CW:(pv.ch + 1) * CW])

        prev2 = None
        for ch in range(NCH):
            sl = slice(ch * CW, (ch + 1) * CW)
            pv = Prev()
            pv.ch = ch
            pv.e8 = e8p.tile([P, SB, CW], FP8, name="e8")
            pv.avT = avp.tile([P, DC, CW], FP8, name="avT")
            e8 = pv.e8
            dps = psd.tile([1, CW], FP32, name="dps", tag="psd")
            for skb in range(SB):
                ksl = slice(skb * P, (skb + 1) * P)
                scp = ps.tile([P, CW], FP32, name="scps", tag="ps")
                for c2 in range(DC // 2):
                    nc.tensor.matmul(scp, kT[:, 2 * c2:2 * c2 + 2, ksl],
                                     qT[:, 2 * c2:2 * c2 + 2, sl],
                                     start=(c2 == 0), stop=(c2 == DC // 2 - 1),
                                     perf_mode=DR)
                nc.scalar.activation(e8[:, skb, :], scp, AF.Exp,
                                     scale=1.0 / D)
                if skb % 2 == 1:
                    nc.tensor.matmul(dps, ones8p[:, :, 0:1],
                                     e8[:, skb - 1:skb + 1, :],
                                     start=(skb == 1), stop=(skb == SB - 1),
                                     skip_group_check=True, perf_mode=DR)
                if prev2 is not None:
                    if skb == 1:
                        tail2(prev2, 0)
                    elif skb == 6:
                        tail2(prev2, 1)
                    elif skb == 12:
                        tail2(prev2, 2)
                if prev is not None:
                    prev_tail(prev, skb)
            # denominator reciprocal for this chunk (DVE; consumed by the
            # tail steps that run inside the next chunk's scores loop)
            rec = small.tile([1, CW], FP32, name="rec", bufs=1)
            pv.rec16 = small.tile([1, CW], BF16, name="rec16", bufs=1)
            nc.vector.reciprocal(rec, dps)
            nc.vector.tensor_copy(pv.rec16, rec)
            prev2 = prev
            prev = pv

        def mlp1_group(g_t, ch, hm):
            sl = slice(ch * CW, (ch + 1) * CW)
            hsl = slice(hm * P, (hm + 1) * P)
            mp = psm.tile([P, CW], FP32, name="mps", tag="psm")
            for c in range(DC):
                nc.tensor.matmul(mp, w1_sb[:, c, hsl],
                                 h2_all[:, c, sl],
                                 start=(c == 0), stop=(c == DC - 1))
            gi = nc.scalar.activation(g_t[:, hm, :], mp, AF.Gelu)
            if act_a_last[0] is not None and OPTS.get("pin_gelu", True):
                tile.add_dep_helper(gi.ins, act_a_last[0].ins,
                                    reason="act table phase boundary")

        # epilogue: remaining tails, attnV(3) interleaved with the ln2
        # chains of chunks 2 and 3; two MLP1 groups of chunk 0 pre-emitted
        # as extra PE filler (their gelus stay pinned behind phase A)
        g_t0 = gtp.tile([P, HC, CW], BF16, name="g_t")
        tail2(prev2, 0)
        prev_tail(prev, 2)
        prev_tail(prev, 3)
        for step in range(4, 7):
            prev_tail(prev, step)
        tail2(prev2, 1)
        mlp1_group(g_t0, 0, 0)
        for step in range(7, 11):
            prev_tail(prev, step)
        tail2(prev2, 2)
        mlp1_group(g_t0, 0, 1)
        prev_tail(prev, 11)
        prev_tail(prev, 14)
        prev_tail(prev, 15)
        for step in range(3):
            tail2(prev, step)

        # ================= phase B: MLPs (gelu table) =================
        # pin the ACT stream: every gelu must follow the last phase-A ACT op
        # (rstd of chunk 3), so the act table switches exactly once per phase
        # even though the scheduler hoists MLP matmuls into attention bubbles.
        for ch in range(NCH):
            sl = slice(ch * CW, (ch + 1) * CW)
            g_t = g_t0 if ch == 0 else gtp.tile([P, HC, CW], BF16, name="g_t")
            for hm in range(2 if ch == 0 else 0, HC):
                mlp1_group(g_t, ch, hm)
            for half in range(2):
                m2 = psb.tile([P, 2, CW], FP32, name="m2ps", tag="psb")
                for mh in range(2):
                    m = 2 * half + mh
                    msl = slice(m * P, (m + 1) * P)
                    for hm in range(HC):
                        nc.tensor.matmul(m2[:, mh, :], w2_sb[:, hm, msl],
                                         g_t[:, hm, :],
                                         start=(hm == 0), stop=(hm == HC - 1),
                                         skip_group_check=True)
                for mh in range(2):
                    m = 2 * half + mh
                    if chain:
                        nc.vector.tensor_add(x_sb[:, m, sl],
                                             x_sb[:, m, sl], m2[:, mh, :])
                    else:
                        o_t = small.tile([P, CW], FP32, name="ot", bufs=2)
                        nc.vector.tensor_add(o_t, x_sb[:, m, sl],
                                             m2[:, mh, :])
                        nc.sync.dma_start(
                            t["outT"].rearrange("(c p) s -> p c s",
                                                p=P)[:, m, sl],
                            o_t)

    if repeat is None:
        one_block(chain=False)
    elif repeat == 0:
        one_block(chain=True)
        for ch in range(NCH):
            sl = slice(ch * CW, (ch + 1) * CW)
            nc.sync.dma_start(
                t["outT"].rearrange("(c p) s -> p c s", p=P)[:, :, sl],
                x_sb[:, :, sl])
    else:
        with tc.For_i(0, repeat, 1):
            one_block(chain=True)
        nc.sync.dma_start(t["outT"].rearrange("(c p) s -> p c s", p=P), x_sb)


_CACHE = {}


def _build(S, D, H, repeat=None, trivial_gb=False):
    key = (S, D, H, repeat, trivial_gb, tuple(sorted(OPTS.items())))
    if key in _CACHE:
        return _CACHE[key]
    nc = bacc.Bacc("TRN2", target_bir_lowering=False, debug=False,
                   num_devices=N_CORES)
    t = {}
    t["xT"] = nc.dram_tensor("xT", [D, S], FP32, kind="ExternalInput").ap()
    for w, shp in (("wq", [D, D]), ("wk", [D, D]), ("wv", [D, D]),
                   ("wp", [D, D])):
        t[w] = nc.dram_tensor(w, shp, FP8, kind="ExternalInput").ap()
    for w, shp in (("w1", [D, H]), ("w2", [H, D])):
        t[w] = nc.dram_tensor(w, shp, BF16, kind="ExternalInput").ap()
    for g in ("g1", "b1", "g2", "b2"):
        t[g] = nc.dram_tensor(g, [D], FP32, kind="ExternalInput").ap()
    t["outT"] = nc.dram_tensor("outT", [D, S], FP32, kind="ExternalOutput").ap()

    with tile.TileContext(nc) as tc:
        _block_kernel(tc, t, S, D, H, repeat=repeat, trivial_gb=trivial_gb)
    nc.compile()
    _CACHE[key] = nc
    return nc


def _in_maps(x, wq, wk, wv, wp, w1, w2, g1, b1, g2, b2):
    bf = ml_dtypes.bfloat16
    f8 = ml_dtypes.float8_e4m3
    shared = {
        "wq": np.ascontiguousarray(np.asarray(wq, dtype=np.float32).astype(f8)),
        "wk": np.ascontiguousarray(np.asarray(wk, dtype=np.float32).astype(f8)),
        "wv": np.ascontiguousarray(np.asarray(wv, dtype=np.float32).astype(f8)),
        "wp": np.ascontiguousarray(np.asarray(wp, dtype=np.float32).astype(f8)),
        "w1": np.ascontiguousarray(np.asarray(w1, dtype=np.float32).astype(bf)),
        "w2": np.ascontiguousarray(np.asarray(w2, dtype=np.float32).astype(bf)),
        "g1": np.ascontiguousarray(g1, dtype=np.float32),
        "b1": np.ascontiguousarray(b1, dtype=np.float32),
        "g2": np.ascontiguousarray(g2, dtype=np.float32),
        "b2": np.ascontiguousarray(b2, dtype=np.float32),
    }
    maps = []
    for i in range(N_CORES):
        m = dict(shared)
        m["xT"] = np.ascontiguousarray(np.asarray(x[i], dtype=np.float32).T)
        maps.append(m)
    return maps


def _gb_trivial(g1, b1, g2, b2):
    return (np.all(np.asarray(g1) == 1.0) and np.all(np.asarray(b1) == 0.0)
            and np.all(np.asarray(g2) == 1.0) and np.all(np.asarray(b2) == 0.0))


def run(x, wq, wk, wv, wp, w1, w2, g1, b1, g2, b2, repeat=None, **kwargs):
    """Build + run on 8 cores; returns (output [B,S,D], BassKernelResults)."""
    x = np.asarray(x)
    B, S, D = x.shape
    H = np.asarray(w1).shape[1]
    assert B == N_CORES
    nc = _build(S, D, H, repeat=repeat,
                trivial_gb=_gb_trivial(g1, b1, g2, b2))
    maps = _in_maps(x, wq, wk, wv, wp, w1, w2, g1, b1, g2, b2)
    res = run_bass_kernel_spmd(nc, maps, core_ids=list(range(N_CORES)), **kwargs)
    out = np.empty((B, S, D), dtype=np.float32)
    for i in range(N_CORES):
        out[i] = res.results[i]["outT"].T
    return out, res


def kernel(x, wq, wk, wv, wp, w1, w2, g1, b1, g2, b2):
    out, _ = run(x, wq, wk, wv, wp, w1, w2, g1, b1, g2, b2)
    return out
